# revision 1
# baseline (speedup 1.0000x reference)
"""2-layer multi-head GAT on 8 Trainium2 NeuronCores (Bass/Tile, single launch).

v2: bf16 gather tables (256B rows), host-precomputed bf16 one-hot/window
masks streamed from HBM via HWDGE (no on-chip mask generation), all-bf16
matmuls (FWL), single-bf16 adst stats (no hi/lo split), epilogues batched
over 7-block groups, Shared-space AllGather outputs.

Distribution: dst-sharded message passing as v1 (see kernel.py docstring).
"""
import math
from contextlib import ExitStack

import numpy as np
import ml_dtypes

import concourse.bass as bass
import concourse.bacc as bacc
import concourse.mybir as mybir
import concourse.tile as tile
import concourse.bass2jax as b2j
from concourse.library_config import mlp
from concourse.tile_rust import add_dep_helper

F32 = mybir.dt.float32
BF16 = mybir.dt.bfloat16
I16 = mybir.dt.int16
I32 = mybir.dt.int32
OP = mybir.AluOpType
AF = mybir.ActivationFunctionType
BF = ml_dtypes.bfloat16

LEAKY = 0.2
PAD_DST = 200
N_CORES = 8
GRP = 7
PREP_GATHER = False
KMAX = 8


def cdiv(a, b):
    return -(-a // b)


def _wrap_idx16(idx):
    n = idx.shape[0]
    w = idx.reshape(n // 16, 16).T.astype(np.int16)
    return np.tile(w, (8, 1))


def host_prep(x, edge_indices, W1, a_src1, a_dst1, b1, W2, a_src2, a_dst2, b2,
              n_cores=N_CORES):
    N, Din = x.shape
    D1 = W1.shape[1]
    H1, Dh1 = a_src1.shape
    D2 = W2.shape[1]
    assert Din == 128 and D1 == 64 and H1 * Dh1 == D1

    SHARD = cdiv(N, n_cores * 128) * 128
    NTOT = SHARD * n_cores
    HALF = NTOT // 2
    B = SHARD // 128
    assert HALF % 128 == 0 and HALF <= 32768 and B % GRP == 0

    A_src = np.zeros((D1, H1), np.float64)
    A_dst = np.zeros((D1, H1), np.float64)
    for h in range(H1):
        A_src[h * Dh1:(h + 1) * Dh1, h] = a_src1[h]
        A_dst[h * Dh1:(h + 1) * Dh1, h] = a_dst1[h]
    W1f = np.concatenate([W1.astype(np.float64),
                          W1.astype(np.float64) @ A_src,
                          W1.astype(np.float64) @ A_dst], axis=1).astype(np.float32)
    w_as2 = (W2.astype(np.float64) @ a_src2[0].astype(np.float64)).astype(np.float32)
    w_ad2 = (W2.astype(np.float64) @ a_dst2[0].astype(np.float64)).astype(np.float32)

    xT = np.zeros((128, NTOT), np.float32)
    xT[:, :N] = x.T

    src = np.asarray(edge_indices[0], np.int64)
    dst = np.asarray(edge_indices[1], np.int64)
    core = dst // SHARD

    percore = []
    nlo = np.zeros((n_cores, B), np.int64)
    nhi = np.zeros((n_cores, B), np.int64)
    for c in range(n_cores):
        m = core == c
        s_c, d_c = src[m], dst[m]
        blk = (d_c - c * SHARD) >> 7
        half = (s_c >= HALF).astype(np.int64)
        order = np.lexsort((d_c, half, blk))
        s_c, d_c, blk, half = s_c[order], d_c[order], blk[order], half[order]
        cnt = np.bincount(blk * 2 + half, minlength=2 * B)
        nlo[c] = cnt[0::2]
        nhi[c] = cnt[1::2]
        percore.append((s_c, d_c, blk, half))

    NLO = np.maximum(cdiv(nlo, 128).max(axis=0), 1)
    NHI = np.maximum(cdiv(nhi, 128).max(axis=0), 1)
    NSUB = int((NLO + NHI).sum())
    TOT = NSUB * 128

    sub_off_lo = np.zeros(B, np.int64)
    sub_off_hi = np.zeros(B, np.int64)
    acc = 0
    for b in range(B):
        sub_off_lo[b] = acc
        acc += NLO[b]
        sub_off_hi[b] = acc
        acc += NHI[b]

    gops = []
    for b in range(B):
        ops = []
        for tbl, off, k in ((0, sub_off_lo[b], NLO[b]), (1, sub_off_hi[b], NHI[b])):
            done = 0
            while done < k:
                take = min(KMAX, k - done)
                ops.append((tbl, int(off + done), int(take)))
                done += take
        gops.append(ops)

    streams = []
    for c in range(n_cores):
        s_c, d_c, blk, half = percore[c]
        idxs = np.zeros(TOT, np.int64)
        dstloc = np.full(TOT, PAD_DST, np.int64)
        pos = np.zeros(len(s_c), np.int64)
        for b in range(B):
            for hv, off_sub, _n in ((0, sub_off_lo[b], nlo[c, b]),
                                    (1, sub_off_hi[b], nhi[c, b])):
                m = (blk == b) & (half == hv)
                k = int(m.sum())
                if k:
                    pos[m] = off_sub * 128 + np.arange(k)
        idxs[pos] = np.where(half == 0, s_c, s_c - HALF)
        dstloc[pos] = d_c - c * SHARD - blk * 128

        # interleaved mask tensor: [128, NSUB, 2, 128] bf16
        #   [:, s, 0, :] = st  (edge-partition one-hot of dst)
        #   [:, s, 1, :] = sn  (dst-partition one-hot of edge)
        slot = np.arange(TOT)
        sub = slot >> 7
        i_in = slot & 127
        valid = dstloc < 128
        msk = np.zeros((128, NSUB, 2, 128), BF)
        msk[i_in[valid], sub[valid], 0, dstloc[valid]] = 1
        msk[dstloc[valid], sub[valid], 1, i_in[valid]] = 1

        streams.append({
            "idx16": _wrap_idx16(idxs.astype(np.int16)),
            "msk": msk.reshape(128, NSUB * 256),
        })

    cfg = dict(N=N, n_cores=n_cores, SHARD=SHARD, NTOT=NTOT, HALF=HALF, B=B,
               NLO=NLO, NHI=NHI, NSUB=NSUB, TOT=TOT, gops=gops,
               H1=H1, Dh1=Dh1, D2=D2)
    consts = dict(
        W1f=W1f,
        b1r=np.tile(b1[None, :], (128, 1)).astype(np.float32),
        wa2r=np.tile(w_as2[None, :], (128, 1)).astype(np.float32),
        wd2r=np.tile(w_ad2[None, :], (128, 1)).astype(np.float32),
        w2b=W2.astype(BF),
        b2c=b2.reshape(D2, 1).astype(np.float32),
    )
    in_maps = []
    for c in range(n_cores):
        m = dict(consts)
        m["xTs"] = np.ascontiguousarray(xT[:, c * SHARD:(c + 1) * SHARD])
        m.update(streams[c])
        in_maps.append(m)
    return cfg, in_maps


def build_nc(cfg, repeat=1):
    n_cores = cfg["n_cores"]
    SHARD, NTOT, HALF, B = cfg["SHARD"], cfg["NTOT"], cfg["HALF"], cfg["B"]
    NSUB, TOT = cfg["NSUB"], cfg["TOT"]
    gops = cfg["gops"]
    D2 = cfg["D2"]
    NLO, NHI = cfg["NLO"], cfg["NHI"]
    NG = B // GRP

    nc = bacc.Bacc("TRN2", target_bir_lowering=False, debug=False,
                   num_devices=n_cores, num_swdge_queues=4)

    din = {}
    for name, shape, dt in [
            ("xTs", [128, SHARD], F32), ("W1f", [128, 80], F32),
            ("b1r", [128, 64], F32), ("wa2r", [128, 64], F32),
            ("wd2r", [128, 64], F32), ("w2b", [64, D2], BF16),
            ("b2c", [D2, 1], F32),
            ("idx16", [128, TOT // 16], I16),
            ("msk", [128, NSUB * 256], BF16)]:
        din[name] = nc.dram_tensor(name, shape, dt, kind="ExternalInput").ap()

    z1shard = nc.dram_tensor("z1shard", [SHARD, 128], BF16).ap()
    z1full = nc.dram_tensor("z1full", [NTOT, 128], BF16,
                            addr_space="Shared").ap()
    h2shard = nc.dram_tensor("h2shard", [SHARD, 128], BF16).ap()
    h2full = nc.dram_tensor("h2full", [NTOT, 128], BF16,
                            addr_space="Shared").ap()
    out2T = nc.dram_tensor("out2T", [D2, SHARD], F32, kind="ExternalOutput").ap()

    with tile.TileContext(nc) as tc, ExitStack() as top:
        nc.gpsimd.load_library(mlp)
        cp = top.enter_context(tc.tile_pool(name="consts", bufs=1))

        xts = cp.tile([128, SHARD], F32)
        w1f = cp.tile([128, 80], F32)
        b1r = cp.tile([128, 64], F32)
        wa2r = cp.tile([128, 64], F32)
        wd2r = cp.tile([128, 64], F32)
        w2b = cp.tile([64, D2], BF16)
        b2c = cp.tile([D2, 1], F32)
        idxs = cp.tile([128, TOT // 16], I16)
        for t, name in [(xts, "xTs"), (w1f, "W1f"), (b1r, "b1r"), (wa2r, "wa2r"),
                        (wd2r, "wd2r"), (w2b, "w2b"), (b2c, "b2c"),
                        (idxs, "idx16")]:
            nc.sync.dma_start(t[:], din[name][:])

        iota_i = cp.tile([128, 128], I32)
        iota_c = cp.tile([128, 1], I32)
        iota = cp.tile([128, 128], F32)
        iotac = cp.tile([128, 1], F32)
        identb = cp.tile([128, 128], BF16)
        nc.gpsimd.iota(iota_i[:], [[1, 128]], base=0, channel_multiplier=0)
        nc.gpsimd.iota(iota_c[:], [[1, 1]], base=0, channel_multiplier=1)
        nc.vector.tensor_copy(iota[:], iota_i[:])
        nc.vector.tensor_copy(iotac[:], iota_c[:])
        nc.vector.tensor_scalar(identb[:], iota[:], iotac[:, :1], None,
                                op0=OP.is_equal)

        adst1p = cp.tile([128, B, 8], BF16)
        adst2p = cp.tile([128, B, 1], BF16)
        qsems = ([nc.alloc_semaphore(f"gq{q}") for q in range(4)]
                 if PREP_GATHER else None)

        for _rep in range(repeat):
            # ---------- phase 1: z-prep ----------
            with tc.tile_pool(name="p1", bufs=3) as p1, \
                 tc.tile_pool(name="p1ps", bufs=2, space="PSUM") as p1ps:
                for t in range(B):
                    pz = p1ps.tile([128, 80], F32, space="PSUM")
                    nc.tensor.matmul(pz[:], xts[:, t * 128:(t + 1) * 128], w1f[:],
                                     start=True, stop=True)
                    zw = p1.tile([128, 128], BF16)
                    nc.vector.tensor_copy(zw[:, 0:80], pz[:])
                    if t < 3:
                        nc.vector.memset(zw[:, 80:128], 0.0)
                    nc.vector.tensor_copy(adst1p[:, t, :], pz[:, 72:80])
                    nc.sync.dma_start(z1shard[t * 128:(t + 1) * 128, :], zw[:])

            ag1 = nc.gpsimd.collective_compute(
                "AllGather", OP.bypass, replica_groups=[list(range(n_cores))],
                ins=[z1shard[:]], outs=[z1full[:]])

            def edge_phase(table, adstp, nst, nh, out_cb, ag_inst):
                with tc.tile_pool(name="zg", bufs=8) as zgp, \
                     tc.tile_pool(name="mk", bufs=8) as mkp, \
                     tc.tile_pool(name="stag", bufs=8) as stp, \
                     tc.tile_pool(name="ee", bufs=8) as eep, \
                     tc.tile_pool(name="pm", bufs=2, space="PSUM") as pmp, \
                     tc.tile_pool(name="pa", bufs=3, space="PSUM") as pap, \
                     tc.tile_pool(name="epi", bufs=2) as epi, \
                     tc.tile_pool(name="eps", bufs=1, space="PSUM") as epips:
                    qn = 0
                    for b in range(B):
                        pmain = pmp.tile([128, nst], F32, space="PSUM")
                        nsub_b = int(NLO[b] + NHI[b])
                        si = 0
                        for (tbl, sub0, k) in gops[b]:
                            zg = zgp.tile([128, KMAX, 128], BF16, tag="zg")
                            tab = (table[0:HALF, :] if tbl == 0
                                   else table[HALF:NTOT, :])
                            if PREP_GATHER:
                                nc.gpsimd.dma_gather(
                                    zg[:, 0:k, :], tab,
                                    idxs[:, sub0 * 8:(sub0 + k) * 8],
                                    k * 128, k * 128, 128,
                                    single_packet=True, queue_num=qn % 4,
                                    prepare_only=True, sem=qsems[qn % 4])
                                g = nc.gpsimd.trigger_dma(count=None,
                                                          queue_num=qn % 4)
                            else:
                                g = nc.gpsimd.dma_gather(
                                    zg[:, 0:k, :], tab,
                                    idxs[:, sub0 * 8:(sub0 + k) * 8],
                                    k * 128, k * 128, 128,
                                    single_packet=True, queue_num=qn % 4)
                            if ag_inst is not None:
                                add_dep_helper(g.ins, ag_inst.ins, sync=True,
                                               reason="gather after allgather")
                            qn += 1
                            mk = mkp.tile([128, KMAX, 2, 128], BF16, tag="mk")
                            msrc = bass.AP(din["msk"].tensor,
                                           din["msk"].offset + sub0 * 256,
                                           [[NSUB * 256, 128], [256, k],
                                            [1, 256]])
                            if qn % 2 == 0:
                                nc.sync.dma_start(mk[:, 0:k, :, :], msrc)
                            else:
                                nc.scalar.dma_start(mk[:, 0:k, :, :], msrc)
                            pa = pap.tile([128, KMAX, nh], F32, space="PSUM",
                                          tag="pa")
                            for s in range(k):
                                nc.tensor.matmul(pa[:, s, :], mk[:, s, 1, :],
                                                 adstp[:, b, :], start=True,
                                                 stop=True)
                            ev = eep.tile([128, KMAX, nh], F32, tag="ev")
                            nc.vector.tensor_tensor(
                                ev[:, 0:k, :], zg[:, 0:k, 64:64 + nh],
                                pa[:, 0:k, 0:nh], op=OP.add)
                            lr = eep.tile([128, KMAX, nh], F32, tag="lr")
                            nc.scalar.activation(lr[:, 0:k, :], ev[:, 0:k, :],
                                                 AF.Prelu, alpha=LEAKY)
                            stag = stp.tile([128, KMAX, nst], BF16, tag="stag")
                            exp_out = bass.AP(
                                stag.tensor, stag[:].offset + 64,
                                [[stag[:].ap[0][0], 128], [nst, k], [1, nh]])
                            nc.scalar.activation(exp_out, lr[:, 0:k, :], AF.Exp)
                            expb = bass.AP(
                                stag.tensor, stag[:].offset + 64,
                                [[stag[:].ap[0][0], 128], [nst, k], [1, nh],
                                 [0, 64 // nh]])
                            nc.vector.tensor_tensor(stag[:, 0:k, 0:64],
                                                    zg[:, 0:k, 0:64], expb,
                                                    op=OP.mult)
                            for s in range(k):
                                nc.tensor.matmul(pmain[:], mk[:, s, 0, :],
                                                 stag[:, s, 0:nst],
                                                 start=(si + s == 0),
                                                 stop=(si + s == nsub_b - 1))
                            si += k
                        out_cb(b, pmain, epi, epips)

            h2w_holder = []
            stage_h = {}

            def epi1(b, pmain, epi, epips):
                g, j = divmod(b, GRP)
                if j == 0:
                    stage_t = epi.tile([128, GRP, 72], F32, tag="stage")
                    stage_h["t"] = stage_t
                stg = stage_h["t"]
                nc.vector.tensor_copy(stg[:, j, :], pmain[:])
                if j < GRP - 1:
                    return
                den = epi.tile([128, GRP, 8], F32, tag="den")
                nc.vector.tensor_scalar(den[:], stg[:, :, 64:72], 1e-16, None,
                                        op0=OP.add)
                rden = epi.tile([128, GRP, 8], F32, tag="rden")
                nc.vector.reciprocal(rden[:], den[:])
                o1 = epi.tile([128, GRP, 64], F32, tag="o1")
                rdb = bass.AP(rden.tensor, rden[:].offset,
                              [[rden[:].ap[0][0], 128], [8, GRP], [1, 8],
                               [0, 8]])
                nc.vector.tensor_tensor(o1[:], stg[:, :, 0:64], rdb,
                                        op=OP.mult)
                x1 = epi.tile([128, GRP, 64], F32, tag="x1")
                b1b = bass.AP(b1r.tensor, b1r[:].offset,
                              [[b1r[:].ap[0][0], 128], [0, GRP], [1, 64]])
                nc.vector.tensor_tensor(x1[:], o1[:], b1b, op=OP.add)
                xm = epi.tile([128, GRP, 64], F32, tag="xm")
                nc.vector.tensor_scalar(xm[:], x1[:], 0.0, None, op0=OP.min)
                u = epi.tile([128, GRP, 64], F32, tag="u")
                nc.scalar.activation(u[:], xm[:], AF.Exp)
                v = epi.tile([128, GRP, 64], F32, tag="v")
                nc.vector.tensor_scalar(v[:], x1[:], 0.0, None, op0=OP.max)
                hw = epi.tile([128, GRP, 64], F32, tag="hw")
                nc.vector.scalar_tensor_tensor(hw[:], u[:], -1.0, v[:],
                                               op0=OP.add, op1=OP.add)
                hrow = epi.tile([128, GRP, 128], BF16, tag="hrow")
                nc.vector.tensor_copy(hrow[:, :, 0:64], hw[:])
                nc.vector.memset(hrow[:, :, 65:128], 0.0)
                tr = epi.tile([128, 64], F32, tag="tr")
                t2 = epi.tile([128, 1], F32, tag="t2")
                for j2 in range(GRP):
                    bb = g * GRP + j2
                    nc.vector.scalar_tensor_tensor(tr[:], hw[:, j2, :], 1.0,
                                                   wa2r[:], op0=OP.mult,
                                                   op1=OP.mult,
                                                   accum_out=t2[:])
                    nc.vector.tensor_copy(hrow[:, j2, 64:65], t2[:])
                    nc.vector.scalar_tensor_tensor(tr[:], hw[:, j2, :], 1.0,
                                                   wd2r[:], op0=OP.mult,
                                                   op1=OP.mult,
                                                   accum_out=t2[:])
                    nc.vector.tensor_copy(adst2p[:, bb, 0:1], t2[:])
                dstap = bass.AP(h2shard.tensor,
                                h2shard.offset + g * GRP * 128 * 128,
                                [[128, 128], [128 * 128, GRP], [1, 128]])
                w = nc.sync.dma_start(dstap, hrow[:])
                h2w_holder.append(w)

            edge_phase(z1full, adst1p, 72, 8, epi1, ag1)

            ag2 = nc.gpsimd.collective_compute(
                "AllGather", OP.bypass, replica_groups=[list(range(n_cores))],
                ins=[h2shard[:]], outs=[h2full[:]])
            for w in h2w_holder:
                add_dep_helper(ag2.ins, w.ins, sync=True,
                               reason="h2 write before ag2")

            stage2_h = {}

            def epi2(b, pmain, epi, epips):
                g, j = divmod(b, GRP)
                if j == 0:
                    stage2_t = epi.tile([128, GRP, 65], F32, tag="stage2")
                    stage2_h["t"] = stage2_t
                stg = stage2_h["t"]
                nc.vector.tensor_copy(stg[:, j, :], pmain[:])
                if j < GRP - 1:
                    return
                den = epi.tile([128, GRP, 1], F32, tag="den2")
                nc.vector.tensor_scalar(den[:], stg[:, :, 64:65], 1e-16, None,
                                        op0=OP.add)
                rden = epi.tile([128, GRP, 1], F32, tag="rden2")
                nc.vector.reciprocal(rden[:], den[:])
                agg = epi.tile([128, GRP, 64], BF16, tag="agg")
                rdb = bass.AP(rden.tensor, rden[:].offset,
                              [[rden[:].ap[0][0], 128], [1, GRP], [0, 64]])
                nc.vector.tensor_tensor(agg[:], stg[:, :, 0:64], rdb,
                                        op=OP.mult)
                aggT = epi.tile([64, GRP * 128], BF16, tag="aggT")
                for j2 in range(GRP):
                    ptr = epips.tile([64, 128], BF16, space="PSUM", tag="ptr")
                    nc.tensor.transpose(ptr[:], agg[:, j2, :], identb[:])
                    nc.vector.tensor_copy(aggT[:, j2 * 128:(j2 + 1) * 128],
                                          ptr[:])
                o2 = epi.tile([D2, GRP * 128], F32, tag="o2")
                for (c0, c1) in ((0, 512), (512, GRP * 128)):
                    po2 = epips.tile([D2, 512], F32, space="PSUM", tag="po2")
                    nc.tensor.matmul(po2[:, 0:c1 - c0], w2b[:],
                                     aggT[:, c0:c1], start=True, stop=True)
                    nc.scalar.activation(o2[:, c0:c1], po2[:, 0:c1 - c0],
                                         AF.Identity, bias=b2c[:, :1])
                nc.sync.dma_start(
                    out2T[:, g * GRP * 128:(g + 1) * GRP * 128], o2[:])

            edge_phase(h2full, adst2p, 65, 1, epi2, ag2)

    nc.compile()
    return nc


class CachedRunner:
    def __init__(self, nc, n_cores):
        import jax
        from jax.sharding import Mesh, PartitionSpec, NamedSharding
        from jax.experimental.shard_map import shard_map
        b2j.install_neuronx_cc_hook()
        self.nc = nc
        self.n_cores = n_cores
        in_names, out_names, out_avals = [], [], []
        for alloc in nc.m.functions[0].allocations:
            if not isinstance(alloc, mybir.MemoryLocationSet):
                continue
            name = alloc.memorylocations[0].name
            if alloc.kind == "ExternalInput":
                if (nc.partition_id_tensor is None
                        or name != nc.partition_id_tensor.name):
                    in_names.append(name)
            elif alloc.kind == "ExternalOutput":
                out_names.append(name)
                out_avals.append(jax.core.ShapedArray(
                    tuple(alloc.tensor_shape), mybir.dt.np(alloc.dtype)))
        self.in_names, self.out_names, self.out_avals = \
            in_names, out_names, out_avals
        n_params = len(in_names)
        all_in = list(in_names) + list(out_names)
        if nc.partition_id_tensor is not None:
            all_in.append(nc.partition_id_tensor.name)

        def _body(*args):
            operands = list(args)
            if nc.partition_id_tensor is not None:
                operands.append(b2j.partition_id_tensor())
            outs = b2j._bass_exec_p.bind(
                *operands, out_avals=tuple(out_avals), in_names=tuple(all_in),
                out_names=tuple(out_names), lowering_input_output_aliases=(),
                sim_require_finite=False, sim_require_nnan=False, nc=nc)
            return tuple(outs)

        self.jax = jax
        self.devices = jax.devices()[:n_cores]
        self.mesh = Mesh(np.asarray(self.devices), ("core",))
        donate = tuple(range(n_params, n_params + len(out_names)))
        self.fn = jax.jit(
            shard_map(_body, mesh=self.mesh,
                      in_specs=(PartitionSpec("core"),) * (n_params +
                                                           len(out_names)),
                      out_specs=(PartitionSpec("core"),) * len(out_names),
                      check_rep=False),
            donate_argnums=donate, keep_unused=True)
        self.sh = NamedSharding(self.mesh, PartitionSpec("core"))
        self.dev_ins = None

    def put_inputs(self, in_maps):
        concat = [np.concatenate([np.asarray(in_maps[c][n])
                                  for c in range(self.n_cores)], axis=0)
                  for n in self.in_names]
        self.dev_ins = [self.jax.device_put(a, self.sh) for a in concat]
        for a in self.dev_ins:
            a.block_until_ready()

    def __call__(self):
        jnp = self.jax.numpy
        zeros = [self.jax.device_put(
            jnp.zeros((self.n_cores * av.shape[0], *av.shape[1:]), av.dtype),
            self.sh) for av in self.out_avals]
        outs = self.fn(*self.dev_ins, *zeros)
        return {name: np.asarray(outs[i]).reshape(
                    self.n_cores, *self.out_avals[i].shape)
                for i, name in enumerate(self.out_names)}


_STATE = {}


def _fingerprint(inputs):
    import hashlib
    h = hashlib.sha256()
    for k in sorted(inputs):
        a = np.asarray(inputs[k])
        h.update(k.encode())
        h.update(str(a.shape).encode())
        h.update(str(a.dtype).encode())
        h.update(np.ascontiguousarray(a).tobytes())
    return h.hexdigest()


def _get_state(inputs, repeat=1):
    fp = _fingerprint(inputs)
    key = (fp, repeat)
    st = _STATE.get("st")
    if st is not None and st["key"] == key:
        return st
    prep = _STATE.get("prep")
    if prep is not None and prep[0] == fp:
        cfg, in_maps = prep[1], prep[2]
    else:
        cfg, in_maps = _host_prep_all(inputs)
        _STATE["prep"] = (fp, cfg, in_maps)
    nc = build_nc(cfg, repeat=repeat)
    runner = CachedRunner(nc, cfg["n_cores"])
    runner.put_inputs(in_maps)
    st = {"key": key, "cfg": cfg, "runner": runner}
    _STATE["st"] = st
    return st


def _host_prep_all(inputs):
    return host_prep(
        np.asarray(inputs["x"], np.float32),
        np.asarray(inputs["edge_indices"]),
        np.asarray(inputs["W1"], np.float32),
        np.asarray(inputs["a_src1"], np.float32),
        np.asarray(inputs["a_dst1"], np.float32),
        np.asarray(inputs["b1"], np.float32),
        np.asarray(inputs["W2"], np.float32),
        np.asarray(inputs["a_src2"], np.float32),
        np.asarray(inputs["a_dst2"], np.float32),
        np.asarray(inputs["b2"], np.float32))


def kernel(**inputs):
    st = _get_state(inputs)
    res = st["runner"]()
    cfg = st["cfg"]
    out = np.concatenate([res["out2T"][c].T for c in range(cfg["n_cores"])],
                         axis=0)[:cfg["N"]]
    return np.ascontiguousarray(out.astype(np.float32))


def measure_device_time_ns(inputs, k0=1, k1=5, reps=10):
    """Slope-based device-time estimate: build K-repeat variants of the whole
    kernel body and difference the best wall times (fixed dispatch cancels)."""
    import time
    times = {}
    saved = _STATE.pop("st", None)
    for K in (k0, k1):
        st = _get_state(inputs, repeat=K)
        st["runner"]()
        st["runner"]()
        best = 1e9
        for _ in range(reps):
            t0 = time.time()
            st["runner"]()
            best = min(best, time.time() - t0)
        times[K] = best
        _STATE.pop("st", None)
    if saved is not None:
        _STATE["st"] = saved
    return (times[k1] - times[k0]) / (k1 - k0) * 1e9

